# revision 77
# baseline (speedup 1.0000x reference)
"""Block-sparse (DeepSpeed fixed-layout) causal self-attention on 8 trn2 NeuronCores.

Problem: B=2, H=16, L=2048, D=64, fp32; BLOCK=16, STRIDE=64, NUMVERTS=1, VERTSIZE=1.
Layout per head (identical for all heads since numverts=1):
  - intra-window block-causal attention within each 64-token window (4 blocks of 16)
  - "summary" attention: every query attends the last 16 tokens (block col 3) of
    every *earlier* 64-token window.

Strategy (per core; 32 (b,h) pairs sharded 4 per core, no collectives).
The Activation engine (exp, ~0.83 ns/column + ~185 ns/instr access penalty)
is the binding resource; the design minimizes exp'd columns and exp
instruction count, and uses a "flipped" AV so outputs land untransposed:

  Summary QK:  St[sk,q] = lhsT.T @ rhs with
     lhsT = [summary K^T ; one-hot selector rows]   (stationary, fp16)
     rhs  = [Q^T/8 ; per-(window,block) mask-value rows]  (moving, fp16)
  -- window-granularity causal masks (-30000) fused into the matmul as
  contraction rows per 128-key chunk.  exp applies bias=-4 (ACT free affine)
  so the unnormalized sums stay in fp16 range; pieces of 2 chunks share one
  fused-AP exp instruction.

  Packed local QK: the two 64-token windows of each window pair are computed
  by 64x64 PE tiles at positions (0,0)/(0,64) -- both read partitions 0-63
  (no second Q/K copy needed), the odd window's scores land at output
  partitions 64-127, packing both windows' scores into one [128, 64] PSUM
  column range and HALVING local exp columns.  Local exp is fused per bh
  (one [128, 4, 256] instruction; per pair on the startup bh where the
  staged DMA wire would stall the in-order ACT queue).  The in-window
  block-causal mask is ONE DVE multiply per group by a 1/0 mask that is
  identical on both partition halves.

  Flipped AV:  O[q, dc] = Et.T @ [V|1] -- the exp'd scores are the *stationary*
  operand ([<=128 keys, <=128 queries] blocks), [V|1] (65 cols) streams as
  the moving operand.  Output accumulates in natural [query, d] layout, one
  [128, 4, 128] PSUM tile (bank) per group; col 64 is the softmax
  denominator l.  The local A/B windows are separated by ROW-TILED matmuls
  -- A contracts only partitions 0-63 (tile (0,0)), B only 64-127 (tile
  (64,64)) -- so no mask-based zeroing of the other half is needed.  Both
  u==0 halves carry start=True since the pending-zero of start only covers
  the instruction's own partition range.

  The whole (b,h) x group unit stream is software-pipelined with depth 1:
  unit t's AV/copy is emitted after unit t+1's QK/exp, so ACT (the binding
  engine, ~77% busy) never waits on the AV chain.  The last bh runs its
  groups in reverse so the kernel tail is the shortest AV chain, and the
  final output halves ship individually (the very last via SP/HWDGE, no
  ~1us SWDGE trigger).

  The device ships unnormalized O rows fp16 (DVE copy PSUM->SBUF, one
  Pool-queue DMA per 2 groups); the host divides: out = O[..., :64]/O[..., 64:].
  Startup: bh0's operands arrive as staged 128-partition pieces of `pre`
  (data + mask rows in one transfer each) ordered by first use on the
  serial DMA wire; PE-clock warmup matmuls run while they are in flight.
"""

import numpy as np

# ---------------- problem constants (hardcoded per contract) ----------------
B, H, L, D = 2, 16, 2048, 64
BLOCK = 16
WIN = 64              # stride window (tokens)
NWIN = L // WIN       # 32 windows
NSUM = NWIN * BLOCK   # 512 summary keys (last 16 tokens of each window)
NG = 4                # query groups per sequence
GQ = L // NG          # 512 queries per group
NCORES = 8
NBH = (B * H) // NCORES  # 4 (b,h) per core
MASKVAL = -30000.0
EXPBIAS = -4.0        # exp(s-4): keeps unnormalized fp16 sums in range

_SUMIDX = np.array([64 * m + 48 + j for m in range(NWIN) for j in range(BLOCK)])


def _host_masks():
    """Constant mask rows appended to the summary-QK contraction dim. fp16.

    mq [64, L]    : mask *values* rows (appended to Q^T, the moving operand)
                    rows 8-39 = per-(chunk s, block b) window-granularity
                    causal values; masked iff summary window m=8s+b >= the
                    query's window (same-window attention is the local path).
    ms [64, NSUM] : selector rows appended to the gathered summary K^T
                    rows 8+8s+b = one-hot of summary chunk s, block b
    """
    qc = np.arange(L)
    mq = np.zeros((64, L), np.float32)
    for s in range(4):
        for b in range(8):
            mq[8 + 8 * s + b] = np.where(8 * s + b >= qc // WIN, MASKVAL, 0.0)
    sc = np.arange(NSUM)
    ms = np.zeros((64, NSUM), np.float32)
    for s in range(4):
        for b in range(8):
            ms[8 + 8 * s + b] = ((sc // 128 == s) & ((sc % 128) // BLOCK == b)).astype(
                np.float32
            )
    return mq.astype(np.float16), ms.astype(np.float16)


def _host_m01():
    """[128, 256] fp16 packed-local block-causal mask (1/0): partition p =
    key offset in window (two windows stacked), col j = (pair-in-group,
    query offset).  Identical pattern on both partition halves; the A/B
    window separation happens in the row-tiled AV matmuls."""
    p = np.arange(128)[:, None]
    j = np.arange(256)[None, :]
    return ((p % 64) // BLOCK <= (j % 64) // BLOCK).astype(np.float16)


# ---------------- device program ----------------
_NC_CACHE = {}

# schedule knobs (fixed; K_* env vars override for tuning experiments)
import os as _os

LAG = int(_os.environ.get("K_LAG", "1"))       # software-pipeline depth
WARM = int(_os.environ.get("K_WARM", "40"))    # PE-clock warmup matmuls


def _build_nc(reps=1):
    """reps>1 repeats the whole computation in-NEFF (timing only)."""
    if reps in _NC_CACHE:
        return _NC_CACHE[reps]
    from contextlib import ExitStack

    import concourse.bacc as bacc
    import concourse.tile as tile
    from concourse import mybir

    F16 = mybir.dt.float16
    F32 = mybir.dt.float32
    EXP = mybir.ActivationFunctionType.Exp

    nc = bacc.Bacc("TRN2", target_bir_lowering=False)

    # qkt = [gathered-summary K^T | Q^T/8 | K^T] concatenated along cols
    # (summary K first so the startup-critical DMA is one contiguous prefix)
    qkt_d = nc.dram_tensor("qkt", [NBH, 64, 2 * L + NSUM], F16, kind="ExternalInput")
    # vpx = gathered summary [V|1] (4 tiles) ++ [V|1] (16 local 128-key
    # tiles) -> one tensor; summary first so bh0's head DMA covers group 0
    vpx_d = nc.dram_tensor("vpx", [NBH, 128, 20, 65], F16, kind="ExternalInput")
    mall_d = nc.dram_tensor("mall", [64, NSUM + L], F16, kind="ExternalInput")
    # bh0 startup copy: [qkt[0][:, 0:NSUM+L] ; mall] stacked to 128 partitions
    # so each staged startup piece is ONE transfer (data + mask rows)
    pre_d = nc.dram_tensor("pre", [128, NSUM + L], F16, kind="ExternalInput")
    m01_d = nc.dram_tensor("m01", [128, 256], F16, kind="ExternalInput")
    # unnormalized output, natural layout: o[i, p, g, b, c] = row 512g+128b+p,
    # col c (c=64 is the softmax denominator l); host divides
    o_d = nc.dram_tensor("o", [NBH, 128, NG, 4, 65], F16, kind="ExternalOutput")

    with tile.TileContext(nc) as tc, ExitStack() as ctx:
        const = ctx.enter_context(tc.tile_pool(name="const", bufs=1))
        inbuf = ctx.enter_context(tc.tile_pool(name="inbuf", bufs=2))
        etp = ctx.enter_context(tc.tile_pool(name="etp", bufs=3))
        etsum = ctx.enter_context(tc.tile_pool(name="etsum", bufs=2))
        psum = ctx.enter_context(tc.tile_pool(name="psum", bufs=2, space="PSUM"))
        outp = ctx.enter_context(tc.tile_pool(name="outp", bufs=2))

        # double-buffered wide base [KTS | Q^T | K^T] with persistent mask rows
        qktb = [const.tile([128, 2 * L + NSUM], F16, name=f"qktb{j}") for j in range(2)]
        m01t = const.tile([128, 256], F16, name="m01t")

        # exp bias (-4.0) as a const AP; memset during startup
        biast = const.tile([128, 1], F32, name="biast")
        nc.gpsimd.memset(biast, EXPBIAS)

        # PE clock warmup: run dummy matmuls on a zeroed scratch tile while
        # the startup DMAs are in flight so the first real matmuls are warm
        # (kept short: the in-order PE queue would otherwise delay the first
        # real matmul past the startup-DMA landing)
        warm = const.tile([128, 64], F16, name="warm")
        nc.any.memzero(warm)
        wps = psum.tile([128, 4, 128], F32, tag="otps", name="warm_ps")
        for w in range(WARM):
            nc.tensor.matmul(
                wps[0:64, 0, 0:64], warm[0:64, :], warm[0:64, :],
                start=True, stop=True, skip_group_check=True,
            )

        def bh_setup(rep, i):
            """Emit input DMAs for (rep, i); return the per-bh view dict."""
            qkt = qktb[i % 2]
            if rep == 0 and i == 0:
                # critical-path startup.  The DMA wire (one shared device in
                # the model) serializes all transfers, and each DMA carries
                # ~2.2us of fixed latency, so bh0's operands arrive as staged
                # 128-partition pieces of `pre` (data + mask rows in one
                # transfer), ordered by first use:
                #   0:1024      KTS + Q group 0 (+ masks)  -> first sQK
                #   1024:1536   Q group 1                  -> sQK g1
                #   kt g0-g1    local QK pair 0
                #   1536:2560   Q groups 2-3
                #   kt rest
                src = qkt_d.ap()[i]
                pre = pre_d.ap()
                nc.sync.dma_start(out=qkt[:, 0:1024], in_=pre[:, 0:1024])
                nc.sync.dma_start(out=qkt[:, 1024:1536], in_=pre[:, 1024:1536])
                nc.sync.dma_start(
                    out=qkt[0:64, NSUM + L : NSUM + L + 1024],
                    in_=src[:, NSUM + L : NSUM + L + 1024],
                )
                nc.scalar.dma_start(out=m01t, in_=m01_d.ap())
                nc.sync.dma_start(out=qkt[:, 1536:2560], in_=pre[:, 1536:2560])
                nc.sync.dma_start(
                    out=qkt[0:64, NSUM + L + 1024 :], in_=src[:, NSUM + L + 1024 :]
                )
            else:
                if rep == 0 and i == 1:
                    # first use of the second buffer: load its mask rows
                    nc.sync.dma_start(
                        out=qkt[64:128, 0 : NSUM + L], in_=mall_d.ap()
                    )
                nc.sync.dma_start(out=qkt[0:64, :], in_=qkt_d.ap()[i])
            vpx = inbuf.tile([128, 20, 65], F16, tag="vpx")
            if rep == 0 and i == 0:
                # group 0's tiles (summary chunk 0 + local pairs 0-3) first
                nc.gpsimd.dma_start(out=vpx[:, 0:8], in_=vpx_d.ap()[i][:, 0:8])
                nc.gpsimd.dma_start(out=vpx[:, 8:20], in_=vpx_d.ap()[i][:, 8:20])
            else:
                nc.gpsimd.dma_start(out=vpx, in_=vpx_d.ap()[i])
            return dict(
                kts=qkt[:, 0:NSUM],
                qt=qkt[:, NSUM : NSUM + L],
                kt=qkt[:, NSUM + L : NSUM + 2 * L],
                vpx=vpx,
            )

        views = {}
        pairs = {}  # (rep, i, j) -> dict(osb=, seen=, masks=)
        locs = {}   # (rep, i) -> dict(stl=, etl=, done=)
        state = {}  # unit (rep, i, g) -> dict(ets=)

        def emit_front(rep, i, g):
            """QK + exp + masks for unit (i, g)."""
            v = views[i]
            kts, qt, kt = v["kts"], v["qt"], v["kt"]
            # ---- summary QK (chunks s = 0..g) + exp fused per 2-chunk
            # piece ----
            ets = []  # per-chunk [128, 512] fp16 APs
            for a in range(0, g + 1, 2):
                b = min(a + 2, g + 1)
                st = psum.tile(
                    [128, 2, GQ], F32, tag="st_sum",
                    name=f"st_{rep}_{i}_{g}_{a}", bufs=2,
                )
                for s in range(a, b):
                    nc.tensor.matmul(
                        st[:, s - a, :],
                        kts[:, 128 * s : 128 * (s + 1)],
                        qt[:, GQ * g : GQ * (g + 1)],
                        start=True,
                        stop=True,
                        skip_group_check=True,
                    )
                e = etsum.tile(
                    [128, 2, GQ], F16, tag="et_sum",
                    name=f"et_{rep}_{i}_{g}_{a}", bufs=6,
                )
                nc.scalar.activation(
                    out=e[:, 0 : b - a, :],
                    in_=st[:, 0 : b - a, :],
                    func=EXP,
                    bias=biast,
                )
                ets.extend(e[:, s - a, :] for s in range(a, b))

            # ---- packed local QK; odd windows go to partitions 64-127 via
            # col-tiled matmuls (tile (0, 64)) so no second Q/K copy is
            # needed.  All 4 groups share one PSUM tile; the exp is fused
            # per bh (one instruction) except on the startup bh, where the
            # wire staggers kt/qt arrival and pair granularity keeps the
            # in-order ACT queue from stalling on late pieces ----
            j = g // 2
            pj = pairs.setdefault((rep, i, j), {})
            if "osb" not in pj:
                pj["osb"] = outp.tile(
                    [128, 2, 4, 65], F16, tag="osb", name=f"osb_{rep}_{i}_{j}",
                )
                pj["seen"] = 0
            li = locs.setdefault((rep, i), {"done": set()})
            first_bh = rep == 0 and i == 0
            # the very first unit defers its local QK/exp to the next front:
            # the ACT queue is in-order, and exp-loc would sit ahead of
            # sQK-g1's exp while waiting on later startup DMA pieces
            defer = first_bh and g == 0 and not li["done"] and "defer" not in li
            if defer:
                li["defer"] = True
            if g not in li["done"] and not defer:
                emit_local(rep, i, (2 * j, 2 * j + 1) if first_bh else (0, 1, 2, 3))

            state[(rep, i, g)] = dict(ets=ets)
            # per-unit masks, emitted in the front as soon as etl exists
            if g in li["done"]:
                for gg in li.pop("pending", []):
                    emit_group_masks(rep, i, gg)
                emit_group_masks(rep, i, g)
            else:
                li.setdefault("pending", []).append(g)

        def emit_local(rep, i, scope):
            """Packed local QK + fused exp for the groups in `scope`."""
            li = locs.setdefault((rep, i), {"done": set()})
            if set(scope) <= li["done"]:
                return
            if "etl" not in li:
                li["stl"] = psum.tile(
                    [128, 4, 256], F32, tag="st_loc", bufs=1,
                    name=f"stl_{rep}_{i}",
                )
                li["etl"] = etp.tile(
                    [128, 4, 256], F16, tag="et_loc", bufs=2,
                    name=f"etl_{rep}_{i}",
                )
            v = views[i]
            kt, qt = v["kt"], v["qt"]
            stl, etl = li["stl"], li["etl"]
            for gg in scope:
                for u in range(4):
                    p = 4 * gg + u
                    nc.tensor.matmul(
                        stl[0:64, gg, 64 * u : 64 * (u + 1)],
                        kt[0:64, 128 * p : 128 * p + 64],
                        qt[0:64, 128 * p : 128 * p + 64],
                        start=True,
                        stop=True,
                        skip_group_check=True,
                    )
                    nc.tensor.matmul(
                        stl[64:128, gg, 64 * u : 64 * (u + 1)],
                        kt[0:64, 128 * p + 64 : 128 * (p + 1)],
                        qt[0:64, 128 * p + 64 : 128 * (p + 1)],
                        start=True,
                        stop=True,
                        skip_group_check=True,
                    )
            lo = min(scope)
            nc.scalar.activation(
                out=etl[:, lo : lo + len(scope), :],
                in_=stl[:, lo : lo + len(scope), :],
                func=EXP,
                bias=biast,
            )
            li["done"].update(scope)

        def emit_group_masks(rep, i, gg):
            """In-window block-causal 1/0 mask (ONE DVE multiply per group).
            The A/B windows are separated by row-tiled AV matmuls, so only
            the causal pattern is applied; it is identical for both
            partition halves."""
            pj = pairs[(rep, i, gg // 2)]
            etl = locs[(rep, i)]["etl"]
            etm = etp.tile(
                [128, 256], F16, tag="et_mska", bufs=4,
                name=f"etm_{rep}_{i}_{gg}",
            )
            nc.vector.tensor_mul(etm, etl[:, gg, :], m01t)
            pj.setdefault("masks", {})[gg] = etm

        def emit_back(rep, i, g, final, ship_now=False):
            """Flipped AV + output copy/DMA for unit (i, g)."""
            u_ = state.pop((rep, i, g))
            ets = u_["ets"]
            vpx = views[i]["vpx"]
            j = g // 2
            pj = pairs[(rep, i, j)]
            etm = pj["masks"][g]
            # ---- flipped AV: O[q, c] accumulated in natural layout, Et
            # blocks stationary, [V|1] (65 cols) moving ----
            ot = psum.tile(
                [128, 4, 128], F32, tag="otps", name=f"ot_{rep}_{i}_{g}", bufs=2,
            )
            for u in range(4):
                # row-tiled halves: A contracts only partitions 0-63
                # (tile (0,0)), B only 64-127 (tile (64,64)) -- the window
                # separation comes from the tiling, not mask zeroing.
                # start=True pending-zeroes the whole bank but only for the
                # instruction's OWN partition range, so both u==0 halves
                # need it; everything after overwrites/accumulates cleared
                # regions with start=False.
                nc.tensor.matmul(
                    ot[0:64, u, 0:65],
                    etm[0:64, 64 * u : 64 * (u + 1)],
                    vpx[0:64, 4 + 4 * g + u, :],
                    start=(u == 0),
                    stop=False,
                    skip_group_check=True,
                )
                nc.tensor.matmul(
                    ot[64:128, u, 0:65],
                    etm[64:128, 64 * u : 64 * (u + 1)],
                    vpx[64:128, 4 + 4 * g + u, :],
                    start=(u == 0),
                    stop=False,
                    skip_group_check=True,
                )
            for s in range(g + 1):
                for u in range(4):
                    nc.tensor.matmul(
                        ot[:, u, 0:65],
                        ets[s][:, 128 * u : 128 * (u + 1)],
                        vpx[:, s, :],
                        start=False,
                        stop=(s == g and u == 3),
                        skip_group_check=True,
                    )

            # ---- move unnormalized O to SBUF fp16 (host divides) ----
            osb = pj["osb"]
            nc.vector.tensor_copy(out=osb[:, g % 2], in_=ot[:, :, 0:65])
            pj["seen"] += 1
            if final or ship_now:
                # tail: ship each of the last pair's halves as soon as its
                # copy lands; the very last goes via SP/HWDGE (no ~1us
                # SWDGE trigger on the critical tail)
                dma_q = nc.sync if final else nc.gpsimd
                dma_q.dma_start(
                    out=o_d.ap()[i][:, 2 * j + g % 2], in_=osb[:, g % 2]
                )
            elif pj["seen"] == 2:
                nc.gpsimd.dma_start(out=o_d.ap()[i][:, 2 * j : 2 * j + 2], in_=osb)

        # software pipeline, depth 2: the AV/copy of unit t is emitted after
        # unit t+2's QK/exp so neither ACT nor the in-order PE queue waits on
        # the AV/mask chain; the last bh runs its groups in reverse so the
        # kernel tail is the shortest AV chain (group 0).
        stream = []
        for rep in range(reps):
            for i in range(NBH):
                gs = range(NG) if i < NBH - 1 else reversed(range(NG))
                stream.extend((rep, i, g) for g in gs)
        n = len(stream)
        done_setup = set()

        def ensure_setup(rep, i):
            if (rep, i) not in done_setup:
                done_setup.add((rep, i))
                views[i] = bh_setup(rep, i)

        for t, (rep, i, g) in enumerate(stream):
            ensure_setup(rep, i)
            emit_front(rep, i, g)
            if t >= LAG:
                b = stream[t - LAG]
                emit_back(b[0], b[1], b[2], final=False, ship_now=(b == stream[n - 2]))
        for t in range(max(0, n - LAG), n):
            b = stream[t]
            emit_back(
                b[0], b[1], b[2],
                final=(t == n - 1),
                ship_now=(t == n - 2),
            )

    nc.compile()
    _NC_CACHE[reps] = nc
    return nc


def _prep_core_inputs(qf, kf, vf, bhs, mq, ms):
    """Build one core's input dict from flat [32, L, D] fp32 arrays."""
    qkt = np.empty((NBH, 64, 2 * L + NSUM), np.float16)
    vpx = np.empty((NBH, 128, 20, 65), np.float16)
    for j, bh in enumerate(bhs):
        qkt[j, :, 0:NSUM] = kf[bh][_SUMIDX].T.astype(np.float16)
        qkt[j, :, NSUM : NSUM + L] = (qf[bh].T * 0.125).astype(np.float16)
        qkt[j, :, NSUM + L :] = kf[bh].T.astype(np.float16)
        vp1 = np.concatenate([vf[bh], np.ones((L, 1), np.float32)], axis=1).astype(
            np.float16
        )
        vpx[j, :, 4:, :] = vp1.reshape(16, 128, 65).transpose(1, 0, 2)
        vs1 = np.concatenate(
            [vf[bh][_SUMIDX], np.ones((NSUM, 1), np.float32)], axis=1
        ).astype(np.float16)
        vpx[j, :, :4, :] = vs1.reshape(4, 128, 65).transpose(1, 0, 2)
    mall = np.concatenate([ms, mq], axis=1)
    pre = np.concatenate([qkt[0, :, 0 : NSUM + L], mall], axis=0)
    return {"qkt": qkt, "vpx": vpx, "mall": mall, "pre": pre, "m01": _host_m01()}


def _finish(o_raw):
    """[n, 128, NG, 4, 65] unnormalized device output -> [n, L, 64]."""
    o_raw = np.asarray(o_raw, np.float32)
    o = o_raw[..., :64] / o_raw[..., 64:65]
    # o[i, p, g, b, d] -> out[i, 512g + 128b + p, d]
    return o.transpose(0, 2, 3, 1, 4).reshape(-1, L, 64)


def _in_maps(query, key, value):
    qf = np.asarray(query, np.float32).reshape(B * H, L, D)
    kf = np.asarray(key, np.float32).reshape(B * H, L, D)
    vf = np.asarray(value, np.float32).reshape(B * H, L, D)
    mq, ms = _host_masks()
    return [
        _prep_core_inputs(qf, kf, vf, range(NBH * c, NBH * (c + 1)), mq, ms)
        for c in range(NCORES)
    ]


def kernel(query, key, value):
    from concourse.bass_utils import run_bass_kernel_spmd

    nc = _build_nc()
    res = run_bass_kernel_spmd(nc, _in_maps(query, key, value), list(range(NCORES)))
    out = np.concatenate([_finish(res.results[c]["o"]) for c in range(NCORES)])
    return out.reshape(B, H, L, D).astype(np.float32)


# revision 78
# speedup vs baseline: 1.0009x; 1.0009x over previous
"""Block-sparse (DeepSpeed fixed-layout) causal self-attention on 8 trn2 NeuronCores.

Problem: B=2, H=16, L=2048, D=64, fp32; BLOCK=16, STRIDE=64, NUMVERTS=1, VERTSIZE=1.
Layout per head (identical for all heads since numverts=1):
  - intra-window block-causal attention within each 64-token window (4 blocks of 16)
  - "summary" attention: every query attends the last 16 tokens (block col 3) of
    every *earlier* 64-token window.

Strategy (per core; 32 (b,h) pairs sharded 4 per core, no collectives).
The Activation engine (exp, ~0.83 ns/column + ~185 ns/instr access penalty)
is the binding resource; the design minimizes exp'd columns and exp
instruction count, and uses a "flipped" AV so outputs land untransposed:

  Summary QK:  St[sk,q] = lhsT.T @ rhs with
     lhsT = [summary K^T ; one-hot selector rows]   (stationary, fp16)
     rhs  = [Q^T/8 ; per-(window,block) mask-value rows]  (moving, fp16)
  -- window-granularity causal masks (-30000) fused into the matmul as
  contraction rows per 128-key chunk.  exp applies bias=-4 (ACT free affine)
  so the unnormalized sums stay in fp16 range; pieces of 2 chunks share one
  fused-AP exp instruction.

  Packed local QK: the two 64-token windows of each window pair are computed
  by 64x64 PE tiles at positions (0,0)/(0,64) -- both read partitions 0-63
  (no second Q/K copy needed), the odd window's scores land at output
  partitions 64-127, packing both windows' scores into one [128, 64] PSUM
  column range and HALVING local exp columns.  Local exp is fused per bh
  (one [128, 4, 256] instruction; per pair on the startup bh where the
  staged DMA wire would stall the in-order ACT queue).  The in-window
  block-causal mask is ONE DVE multiply per group by a 1/0 mask that is
  identical on both partition halves.

  Flipped AV:  O[q, dc] = Et.T @ [V|1] -- the exp'd scores are the *stationary*
  operand ([<=128 keys, <=128 queries] blocks), [V|1] (65 cols) streams as
  the moving operand.  Output accumulates in natural [query, d] layout, one
  [128, 4, 128] PSUM tile (bank) per group; col 64 is the softmax
  denominator l.  The local A/B windows are separated by ROW-TILED matmuls
  -- A contracts only partitions 0-63 (tile (0,0)), B only 64-127 (tile
  (64,64)) -- so no mask-based zeroing of the other half is needed.  Both
  u==0 halves carry start=True since the pending-zero of start only covers
  the instruction's own partition range.

  The whole (b,h) x group unit stream is software-pipelined with depth 1:
  unit t's AV/copy is emitted after unit t+1's QK/exp, so ACT (the binding
  engine, ~77% busy) never waits on the AV chain.  The last bh runs its
  groups in reverse so the kernel tail is the shortest AV chain, and the
  final output halves ship individually (the very last via SP/HWDGE, no
  ~1us SWDGE trigger).

  The device ships unnormalized O rows fp16 (DVE copy PSUM->SBUF, one
  Pool-queue DMA per 2 groups); the host divides: out = O[..., :64]/O[..., 64:].
  Startup: bh0's operands arrive as staged 128-partition pieces of `pre`
  (data + mask rows in one transfer each) ordered by first use on the
  serial DMA wire; PE-clock warmup matmuls run while they are in flight.
"""

import numpy as np

# ---------------- problem constants (hardcoded per contract) ----------------
B, H, L, D = 2, 16, 2048, 64
BLOCK = 16
WIN = 64              # stride window (tokens)
NWIN = L // WIN       # 32 windows
NSUM = NWIN * BLOCK   # 512 summary keys (last 16 tokens of each window)
NG = 4                # query groups per sequence
GQ = L // NG          # 512 queries per group
NCORES = 8
NBH = (B * H) // NCORES  # 4 (b,h) per core
MASKVAL = -30000.0
EXPBIAS = -4.0        # exp(s-4): keeps unnormalized fp16 sums in range

_SUMIDX = np.array([64 * m + 48 + j for m in range(NWIN) for j in range(BLOCK)])


def _host_masks():
    """Constant mask rows appended to the summary-QK contraction dim. fp16.

    mq [64, L]    : mask *values* rows (appended to Q^T, the moving operand)
                    rows 8-39 = per-(chunk s, block b) window-granularity
                    causal values; masked iff summary window m=8s+b >= the
                    query's window (same-window attention is the local path).
    ms [64, NSUM] : selector rows appended to the gathered summary K^T
                    rows 8+8s+b = one-hot of summary chunk s, block b
    """
    qc = np.arange(L)
    mq = np.zeros((64, L), np.float32)
    for s in range(4):
        for b in range(8):
            mq[8 + 8 * s + b] = np.where(8 * s + b >= qc // WIN, MASKVAL, 0.0)
    sc = np.arange(NSUM)
    ms = np.zeros((64, NSUM), np.float32)
    for s in range(4):
        for b in range(8):
            ms[8 + 8 * s + b] = ((sc // 128 == s) & ((sc % 128) // BLOCK == b)).astype(
                np.float32
            )
    return mq.astype(np.float16), ms.astype(np.float16)


def _host_m01():
    """[128, 256] fp16 packed-local block-causal mask (1/0): partition p =
    key offset in window (two windows stacked), col j = (pair-in-group,
    query offset).  Identical pattern on both partition halves; the A/B
    window separation happens in the row-tiled AV matmuls."""
    p = np.arange(128)[:, None]
    j = np.arange(256)[None, :]
    return ((p % 64) // BLOCK <= (j % 64) // BLOCK).astype(np.float16)


# ---------------- device program ----------------
_NC_CACHE = {}

# schedule knobs (fixed; K_* env vars override for tuning experiments)
import os as _os

LAG = int(_os.environ.get("K_LAG", "1"))       # software-pipeline depth
WARM = int(_os.environ.get("K_WARM", "40"))    # PE-clock warmup matmuls


def _build_nc(reps=1):
    """reps>1 repeats the whole computation in-NEFF (timing only)."""
    if reps in _NC_CACHE:
        return _NC_CACHE[reps]
    from contextlib import ExitStack

    import concourse.bacc as bacc
    import concourse.tile as tile
    from concourse import mybir

    F16 = mybir.dt.float16
    F32 = mybir.dt.float32
    EXP = mybir.ActivationFunctionType.Exp

    nc = bacc.Bacc("TRN2", target_bir_lowering=False)

    # qkt = [gathered-summary K^T | Q^T/8 | K^T] concatenated along cols
    # (summary K first so the startup-critical DMA is one contiguous prefix)
    qkt_d = nc.dram_tensor("qkt", [NBH, 64, 2 * L + NSUM], F16, kind="ExternalInput")
    # vpx = gathered summary [V|1] (4 tiles) ++ [V|1] (16 local 128-key
    # tiles) -> one tensor; summary first so bh0's head DMA covers group 0
    vpx_d = nc.dram_tensor("vpx", [NBH, 128, 20, 65], F16, kind="ExternalInput")
    mall_d = nc.dram_tensor("mall", [64, NSUM + L], F16, kind="ExternalInput")
    # bh0 startup copy: [qkt[0][:, 0:NSUM+L] ; mall] stacked to 128 partitions
    # so each staged startup piece is ONE transfer (data + mask rows)
    pre_d = nc.dram_tensor("pre", [128, NSUM + L], F16, kind="ExternalInput")
    m01_d = nc.dram_tensor("m01", [128, 256], F16, kind="ExternalInput")
    # unnormalized output, natural layout: o[i, p, g, b, c] = row 512g+128b+p,
    # col c (c=64 is the softmax denominator l); host divides
    o_d = nc.dram_tensor("o", [NBH, 128, NG, 4, 65], F16, kind="ExternalOutput")

    with tile.TileContext(nc) as tc, ExitStack() as ctx:
        const = ctx.enter_context(tc.tile_pool(name="const", bufs=1))
        inbuf = ctx.enter_context(tc.tile_pool(name="inbuf", bufs=2))
        etp = ctx.enter_context(tc.tile_pool(name="etp", bufs=3))
        etsum = ctx.enter_context(tc.tile_pool(name="etsum", bufs=2))
        psum = ctx.enter_context(tc.tile_pool(name="psum", bufs=2, space="PSUM"))
        outp = ctx.enter_context(tc.tile_pool(name="outp", bufs=2))

        # double-buffered wide base [KTS | Q^T | K^T] with persistent mask rows
        qktb = [const.tile([128, 2 * L + NSUM], F16, name=f"qktb{j}") for j in range(2)]
        m01t = const.tile([128, 256], F16, name="m01t")

        # exp bias (-4.0) as a const AP; memset during startup
        biast = const.tile([128, 1], F32, name="biast")
        nc.gpsimd.memset(biast, EXPBIAS)

        # PE clock warmup: run dummy matmuls on a zeroed scratch tile while
        # the startup DMAs are in flight so the first real matmuls are warm
        # (kept short: the in-order PE queue would otherwise delay the first
        # real matmul past the startup-DMA landing)
        warm = const.tile([128, 64], F16, name="warm")
        nc.any.memzero(warm)
        wps = psum.tile([128, 4, 128], F32, tag="otps", name="warm_ps")
        for w in range(WARM):
            nc.tensor.matmul(
                wps[0:64, 0, 0:64], warm[0:64, :], warm[0:64, :],
                start=True, stop=True, skip_group_check=True,
            )

        def bh_setup(rep, i):
            """Emit input DMAs for (rep, i); return the per-bh view dict."""
            qkt = qktb[i % 2]
            if rep == 0 and i == 0:
                # critical-path startup.  The DMA wire (one shared device in
                # the model) serializes all transfers, and each DMA carries
                # ~2.2us of fixed latency, so bh0's operands arrive as staged
                # 128-partition pieces of `pre` (data + mask rows in one
                # transfer), ordered by first use:
                #   0:1024      KTS + Q group 0 (+ masks)  -> first sQK
                #   1024:1536   Q group 1                  -> sQK g1
                #   kt g0-g1    local QK pair 0
                #   1536:2560   Q groups 2-3
                #   kt rest
                src = qkt_d.ap()[i]
                pre = pre_d.ap()
                nc.sync.dma_start(out=qkt[:, 0:1024], in_=pre[:, 0:1024])
                nc.sync.dma_start(out=qkt[:, 1024:1536], in_=pre[:, 1024:1536])
                nc.sync.dma_start(
                    out=qkt[0:64, NSUM + L : NSUM + L + 1024],
                    in_=src[:, NSUM + L : NSUM + L + 1024],
                )
                nc.scalar.dma_start(out=m01t, in_=m01_d.ap())
                nc.sync.dma_start(out=qkt[:, 1536:2560], in_=pre[:, 1536:2560])
                nc.sync.dma_start(
                    out=qkt[0:64, NSUM + L + 1024 :], in_=src[:, NSUM + L + 1024 :]
                )
            else:
                if rep == 0 and i == 1:
                    # first use of the second buffer: load its mask rows
                    nc.sync.dma_start(
                        out=qkt[64:128, 0 : NSUM + L], in_=mall_d.ap()
                    )
                nc.sync.dma_start(out=qkt[0:64, :], in_=qkt_d.ap()[i])
            vpx = inbuf.tile([128, 20, 65], F16, tag="vpx")
            if rep == 0 and i == 0:
                # group 0's tiles (summary chunk 0 + local pairs 0-3) first
                nc.gpsimd.dma_start(out=vpx[:, 0:8], in_=vpx_d.ap()[i][:, 0:8])
                nc.gpsimd.dma_start(out=vpx[:, 8:20], in_=vpx_d.ap()[i][:, 8:20])
            else:
                nc.gpsimd.dma_start(out=vpx, in_=vpx_d.ap()[i])
            return dict(
                kts=qkt[:, 0:NSUM],
                qt=qkt[:, NSUM : NSUM + L],
                kt=qkt[:, NSUM + L : NSUM + 2 * L],
                vpx=vpx,
            )

        views = {}
        pairs = {}  # (rep, i, j) -> dict(osb=, seen=, masks=)
        locs = {}   # (rep, i) -> dict(stl=, etl=, done=)
        state = {}  # unit (rep, i, g) -> dict(ets=)

        def emit_front(rep, i, g):
            """QK + exp + masks for unit (i, g)."""
            v = views[i]
            kts, qt, kt = v["kts"], v["qt"], v["kt"]
            # ---- summary QK (chunks s = 0..g) + exp fused per 2-chunk
            # piece ----
            ets = []  # per-chunk [128, 512] fp16 APs
            for a in range(0, g + 1, 2):
                b = min(a + 2, g + 1)
                st = psum.tile(
                    [128, 2, GQ], F32, tag="st_sum",
                    name=f"st_{rep}_{i}_{g}_{a}", bufs=2,
                )
                for s in range(a, b):
                    nc.tensor.matmul(
                        st[:, s - a, :],
                        kts[:, 128 * s : 128 * (s + 1)],
                        qt[:, GQ * g : GQ * (g + 1)],
                        start=True,
                        stop=True,
                        skip_group_check=True,
                    )
                e = etsum.tile(
                    [128, 2, GQ], F16, tag="et_sum",
                    name=f"et_{rep}_{i}_{g}_{a}", bufs=6,
                )
                nc.scalar.activation(
                    out=e[:, 0 : b - a, :],
                    in_=st[:, 0 : b - a, :],
                    func=EXP,
                    bias=biast,
                )
                ets.extend(e[:, s - a, :] for s in range(a, b))

            # ---- packed local QK; odd windows go to partitions 64-127 via
            # col-tiled matmuls (tile (0, 64)) so no second Q/K copy is
            # needed.  All 4 groups share one PSUM tile; the exp is fused
            # per bh (one instruction) except on the startup bh, where the
            # wire staggers kt/qt arrival and pair granularity keeps the
            # in-order ACT queue from stalling on late pieces ----
            j = g // 2
            pj = pairs.setdefault((rep, i, j), {})
            if "osb" not in pj:
                pj["osb"] = outp.tile(
                    [128, 2, 4, 65], F16, tag="osb", name=f"osb_{rep}_{i}_{j}",
                )
                pj["seen"] = 0
            li = locs.setdefault((rep, i), {"done": set()})
            first_bh = rep == 0 and i == 0
            # the very first unit defers its local QK/exp to the next front:
            # the ACT queue is in-order, and exp-loc would sit ahead of
            # sQK-g1's exp while waiting on later startup DMA pieces
            defer = first_bh and g == 0 and not li["done"] and "defer" not in li
            if defer:
                li["defer"] = True
            if g not in li["done"] and not defer:
                emit_local(rep, i, (2 * j, 2 * j + 1) if first_bh else (0, 1, 2, 3))

            state[(rep, i, g)] = dict(ets=ets)
            # per-unit masks, emitted in the front as soon as etl exists
            if g in li["done"]:
                for gg in li.pop("pending", []):
                    emit_group_masks(rep, i, gg)
                emit_group_masks(rep, i, g)
            else:
                li.setdefault("pending", []).append(g)

        def emit_local(rep, i, scope):
            """Packed local QK + fused exp for the groups in `scope`."""
            li = locs.setdefault((rep, i), {"done": set()})
            if set(scope) <= li["done"]:
                return
            if "etl" not in li:
                li["stl"] = psum.tile(
                    [128, 4, 256], F32, tag="st_loc", bufs=1,
                    name=f"stl_{rep}_{i}",
                )
                li["etl"] = etp.tile(
                    [128, 4, 256], F16, tag="et_loc", bufs=2,
                    name=f"etl_{rep}_{i}",
                )
            v = views[i]
            kt, qt = v["kt"], v["qt"]
            stl, etl = li["stl"], li["etl"]
            for gg in scope:
                for u in range(4):
                    p = 4 * gg + u
                    nc.tensor.matmul(
                        stl[0:64, gg, 64 * u : 64 * (u + 1)],
                        kt[0:64, 128 * p : 128 * p + 64],
                        qt[0:64, 128 * p : 128 * p + 64],
                        start=True,
                        stop=True,
                        skip_group_check=True,
                    )
                    nc.tensor.matmul(
                        stl[64:128, gg, 64 * u : 64 * (u + 1)],
                        kt[0:64, 128 * p + 64 : 128 * (p + 1)],
                        qt[0:64, 128 * p + 64 : 128 * (p + 1)],
                        start=True,
                        stop=True,
                        skip_group_check=True,
                    )
            lo = min(scope)
            nc.scalar.activation(
                out=etl[:, lo : lo + len(scope), :],
                in_=stl[:, lo : lo + len(scope), :],
                func=EXP,
                bias=biast,
            )
            li["done"].update(scope)

        def emit_group_masks(rep, i, gg):
            """In-window block-causal 1/0 mask (ONE DVE multiply per group).
            The A/B windows are separated by row-tiled AV matmuls, so only
            the causal pattern is applied; it is identical for both
            partition halves."""
            pj = pairs[(rep, i, gg // 2)]
            etl = locs[(rep, i)]["etl"]
            etm = etp.tile(
                [128, 256], F16, tag="et_mska", bufs=4,
                name=f"etm_{rep}_{i}_{gg}",
            )
            nc.vector.tensor_mul(etm, etl[:, gg, :], m01t)
            pj.setdefault("masks", {})[gg] = etm

        def emit_back(rep, i, g, final, ship_now=False):
            """Flipped AV + output copy/DMA for unit (i, g)."""
            u_ = state.pop((rep, i, g))
            ets = u_["ets"]
            vpx = views[i]["vpx"]
            j = g // 2
            pj = pairs[(rep, i, j)]
            etm = pj["masks"][g]
            # ---- flipped AV: O[q, c] accumulated in natural layout, Et
            # blocks stationary, [V|1] (65 cols) moving ----
            ot = psum.tile(
                [128, 4, 128], F32, tag="otps", name=f"ot_{rep}_{i}_{g}", bufs=2,
            )
            for u in range(4):
                # row-tiled halves: A contracts only partitions 0-63
                # (tile (0,0)), B only 64-127 (tile (64,64)) -- the window
                # separation comes from the tiling, not mask zeroing.
                # start=True pending-zeroes the whole bank but only for the
                # instruction's OWN partition range, so both u==0 halves
                # need it; everything after overwrites/accumulates cleared
                # regions with start=False.
                nc.tensor.matmul(
                    ot[0:64, u, 0:65],
                    etm[0:64, 64 * u : 64 * (u + 1)],
                    vpx[0:64, 4 + 4 * g + u, :],
                    start=(u == 0),
                    stop=False,
                    skip_group_check=True,
                )
                nc.tensor.matmul(
                    ot[64:128, u, 0:65],
                    etm[64:128, 64 * u : 64 * (u + 1)],
                    vpx[64:128, 4 + 4 * g + u, :],
                    start=(u == 0),
                    stop=False,
                    skip_group_check=True,
                )
            for s in range(g + 1):
                for u in range(4):
                    nc.tensor.matmul(
                        ot[:, u, 0:65],
                        ets[s][:, 128 * u : 128 * (u + 1)],
                        vpx[:, s, :],
                        start=False,
                        stop=(s == g and u == 3),
                        skip_group_check=True,
                    )

            # ---- move unnormalized O to SBUF fp16 (host divides) ----
            osb = pj["osb"]
            nc.vector.tensor_copy(out=osb[:, g % 2], in_=ot[:, :, 0:65])
            pj["seen"] += 1
            if final or ship_now:
                # tail: ship each of the last pair's halves as soon as its
                # copy lands, both via SP/HWDGE (no ~1us SWDGE trigger on
                # the critical tail)
                dma_q = nc.sync
                dma_q.dma_start(
                    out=o_d.ap()[i][:, 2 * j + g % 2], in_=osb[:, g % 2]
                )
            elif pj["seen"] == 2:
                nc.gpsimd.dma_start(out=o_d.ap()[i][:, 2 * j : 2 * j + 2], in_=osb)

        # software pipeline, depth 2: the AV/copy of unit t is emitted after
        # unit t+2's QK/exp so neither ACT nor the in-order PE queue waits on
        # the AV/mask chain; the last bh runs its groups in reverse so the
        # kernel tail is the shortest AV chain (group 0).
        stream = []
        for rep in range(reps):
            for i in range(NBH):
                gs = range(NG) if i < NBH - 1 else reversed(range(NG))
                stream.extend((rep, i, g) for g in gs)
        n = len(stream)
        done_setup = set()

        def ensure_setup(rep, i):
            if (rep, i) not in done_setup:
                done_setup.add((rep, i))
                views[i] = bh_setup(rep, i)

        for t, (rep, i, g) in enumerate(stream):
            ensure_setup(rep, i)
            emit_front(rep, i, g)
            if t >= LAG:
                b = stream[t - LAG]
                emit_back(b[0], b[1], b[2], final=False, ship_now=(b == stream[n - 2]))
        for t in range(max(0, n - LAG), n):
            b = stream[t]
            emit_back(
                b[0], b[1], b[2],
                final=(t == n - 1),
                ship_now=(t == n - 2),
            )

    nc.compile()
    _NC_CACHE[reps] = nc
    return nc


def _prep_core_inputs(qf, kf, vf, bhs, mq, ms):
    """Build one core's input dict from flat [32, L, D] fp32 arrays."""
    qkt = np.empty((NBH, 64, 2 * L + NSUM), np.float16)
    vpx = np.empty((NBH, 128, 20, 65), np.float16)
    for j, bh in enumerate(bhs):
        qkt[j, :, 0:NSUM] = kf[bh][_SUMIDX].T.astype(np.float16)
        qkt[j, :, NSUM : NSUM + L] = (qf[bh].T * 0.125).astype(np.float16)
        qkt[j, :, NSUM + L :] = kf[bh].T.astype(np.float16)
        vp1 = np.concatenate([vf[bh], np.ones((L, 1), np.float32)], axis=1).astype(
            np.float16
        )
        vpx[j, :, 4:, :] = vp1.reshape(16, 128, 65).transpose(1, 0, 2)
        vs1 = np.concatenate(
            [vf[bh][_SUMIDX], np.ones((NSUM, 1), np.float32)], axis=1
        ).astype(np.float16)
        vpx[j, :, :4, :] = vs1.reshape(4, 128, 65).transpose(1, 0, 2)
    mall = np.concatenate([ms, mq], axis=1)
    pre = np.concatenate([qkt[0, :, 0 : NSUM + L], mall], axis=0)
    return {"qkt": qkt, "vpx": vpx, "mall": mall, "pre": pre, "m01": _host_m01()}


def _finish(o_raw):
    """[n, 128, NG, 4, 65] unnormalized device output -> [n, L, 64]."""
    o_raw = np.asarray(o_raw, np.float32)
    o = o_raw[..., :64] / o_raw[..., 64:65]
    # o[i, p, g, b, d] -> out[i, 512g + 128b + p, d]
    return o.transpose(0, 2, 3, 1, 4).reshape(-1, L, 64)


def _in_maps(query, key, value):
    qf = np.asarray(query, np.float32).reshape(B * H, L, D)
    kf = np.asarray(key, np.float32).reshape(B * H, L, D)
    vf = np.asarray(value, np.float32).reshape(B * H, L, D)
    mq, ms = _host_masks()
    return [
        _prep_core_inputs(qf, kf, vf, range(NBH * c, NBH * (c + 1)), mq, ms)
        for c in range(NCORES)
    ]


def kernel(query, key, value):
    from concourse.bass_utils import run_bass_kernel_spmd

    nc = _build_nc()
    res = run_bass_kernel_spmd(nc, _in_maps(query, key, value), list(range(NCORES)))
    out = np.concatenate([_finish(res.results[c]["o"]) for c in range(NCORES)])
    return out.reshape(B, H, L, D).astype(np.float32)
